# revision 28
# baseline (speedup 1.0000x reference)
"""Trainium2 Bass kernel for nn_Block_11020886082299.

Computes, for x: bool[B, DIM_IN], masks: bool[DIM_IN, DIM_OUT],
thresholds: int32[DIM_OUT]:

    sums[b, o] = sum_i XNOR(x[b, i], masks[i, o])
    out[b, o]  = sums[b, o] > thresholds[o]

Math used on device (all exact in fp32):

    sums = DIM_IN - sx[b] - sm[o] + 2 * (x @ m)      (sx/sm = row/col sums)

Encode the x-side weights as (2x-1) in {-1, +1} and stream the raw mask
bytes (0x00 / 0x01) directly as fp8e4m3 (0x01 == 2^-9 denormal, handled
exactly by the PE):

    psum[b, o] = sum_i (2x-1)*U * m*U = U^2 * (2*mm - sm),   U = 2^-9

so with r2[b, o] = U^2 * (t[o] - DIM_IN + sx[b]):

    out = psum > r2   <=>  2*mm - sm > t - DIM_IN + sx  <=>  sums > t

No elementwise conversion of the 16 MB mask tensor ever happens: the DMA
moves raw bytes, the PE consumes them as fp8.

Sharding: tensor-parallel over DIM_OUT across 8 cores (512 columns each);
x is replicated. This minimizes total HBM traffic (each core reads only
its 2 MB slice of masks).
"""

import os

import numpy as np
import ml_dtypes

BATCH = 64
DIM_IN = 4096
DIM_OUT = 4096
N_CORES = 8
OUT_CHUNK = DIM_OUT // N_CORES  # 512
K_TILES = DIM_IN // 128  # 32
CHUNK = 8  # k-tiles per mask DMA (8 * 64KB = 512KB per transfer)
N_CHUNKS = K_TILES // CHUNK
U2 = 2.0 ** -18  # (2^-9)^2 — scale of all PSUM values

_nc = None
last_results = None


def _build():
    import concourse.mybir as mybir
    from concourse import bacc
    from concourse.tile import TileContext

    FP8 = mybir.dt.float8e4
    F32 = mybir.dt.float32
    nc = bacc.Bacc(None, target_bir_lowering=False, debug=False)

    # Combined weights + masks tensor, partition-major so every DMA
    # descriptor is a contiguous multi-KB run per partition:
    #   bytes [0, 2048):       xt[p, k*64+b] = (2x-1)*U   (x side, host-tiled)
    #   bytes [2048, 18432):   mk[p, k*512+c] = raw mask bytes as fp8
    XT_W = K_TILES * BATCH  # 2048
    mk_d = nc.dram_tensor(
        "mk", [128, XT_W + K_TILES * OUT_CHUNK], FP8, kind="ExternalInput"
    )
    # thresholds chunk, broadcast to BATCH rows on host
    tb_d = nc.dram_tensor("tb", [BATCH, OUT_CHUNK], mybir.dt.int32, kind="ExternalInput")
    out_d = nc.dram_tensor("out", [BATCH, OUT_CHUNK], mybir.dt.uint8, kind="ExternalOutput")

    N_WARM = 5  # matmuls to lift the PE HAM clock gate before real data lands

    with TileContext(nc) as tc:
        with (
            tc.tile_pool(name="const", bufs=1) as cpool,
            tc.tile_pool(name="mkp", bufs=1) as mpool,
            tc.tile_pool(name="ps", bufs=1, space="PSUM") as pspool,
        ):
            # ---- warmup memsets first on gpsimd (they gate the PE start).
            # HAM watches MAC activity, so warmup operands must be nonzero:
            # +1 on the top half of K, -1 on the bottom half, so every
            # accumulation cancels exactly to 0.0 in fp32.
            warm = cpool.tile([128, OUT_CHUNK + BATCH], FP8)
            nc.gpsimd.memset(warm[:64, :OUT_CHUNK], 1.0)
            nc.gpsimd.memset(warm[64:, :OUT_CHUNK], -1.0)
            nc.gpsimd.memset(warm[:, OUT_CHUNK:], 2.0 ** -9)
            # ---- stream [xt | masks] in ramped chunks over the two HWDGE
            # rings (the gpsimd SWDGE ring lags ~2.5us, so it only gets the
            # final chunk).
            K_LO = [0, 2, 5, 9, 14, 20, 26, 32]
            RINGS = ["sync", "scalar", "sync", "scalar", "sync", "scalar", "gpsimd"]
            NCH = len(K_LO) - 1
            bounds = [0] + [XT_W + K_LO[i + 1] * OUT_CHUNK for i in range(NCH)]
            mts = []
            for c in range(NCH):
                mt = mpool.tile(
                    [128, bounds[c + 1] - bounds[c]], FP8, tag=f"mk{c}"
                )
                eng = getattr(nc, RINGS[c])
                eng.dma_start(out=mt[:, :], in_=mk_d[:, bounds[c]:bounds[c + 1]])
                mts.append(mt)
            xt_sb = mts[0]  # xt lives in chunk 0, bytes [0, XT_W)

            ones_f = cpool.tile([128, 1], F32)
            nc.gpsimd.memset(ones_f[:, :], 1.0)
            tb_b = cpool.tile([BATCH, OUT_CHUNK], mybir.dt.int32)
            nc.gpsimd.dma_start(out=tb_b[:, :], in_=tb_d[:, :])

            def rhs_for(k):
                c = next(i for i in range(NCH) if K_LO[i] <= k < K_LO[i + 1])
                off = (XT_W if c == 0 else 0) + (k - K_LO[c]) * OUT_CHUNK
                return mts[c][:, off:off + OUT_CHUNK]

            # ---- sx via on-chip data only: xsum[p, b] = sum_k xt[p, k*64+b]
            # (DVE strided reduce over xt, which is already in SBUF), then a
            # single fp32 ones-matmul reduces over partitions:
            #   psx[b] = sum_p xsum[p, b] = U * (2*sx[b] - DIM_IN)
            xsum = cpool.tile([128, BATCH], F32)
            xt3 = xt_sb[:, :XT_W].rearrange("p (k b) -> p b k", b=BATCH)
            nc.vector.tensor_reduce(
                xsum[:, :], xt3, axis=mybir.AxisListType.X, op=mybir.AluOpType.add
            )
            psx = pspool.tile([BATCH, 1], F32, tag="psx")

            psum = pspool.tile([BATCH, OUT_CHUNK], F32)
            for w in range(N_WARM):
                nc.tensor.matmul(
                    psum[:, :], warm[:, OUT_CHUNK:], warm[:, :OUT_CHUNK],
                    start=(w == 0), stop=False, skip_group_check=True,
                )
            for k in range(K_TILES):
                nc.tensor.matmul(
                    psum[:, :],
                    xt_sb[:, k * BATCH:(k + 1) * BATCH],
                    rhs_for(k),
                    start=False,
                    stop=(k == K_TILES - 1),
                )
                if k == 12:
                    nc.tensor.matmul(
                        psx[:, :], xsum[:, :], ones_f[:, :], start=True, stop=True
                    )

            # sxb = U^2*(sx - DIM_IN);  r2 = U^2*t + sxb — ready mid-stream
            sxb = cpool.tile([BATCH, 1], F32)
            nc.vector.tensor_scalar(
                sxb[:, :], psx[:, :], 2.0 ** -10, -float(DIM_IN) / 2.0 * U2,
                mybir.AluOpType.mult, mybir.AluOpType.add,
            )
            r2 = cpool.tile([BATCH, OUT_CHUNK], F32)
            nc.vector.tensor_scalar(
                r2[:, :], tb_b[:, :], U2, sxb[:, 0:1],
                mybir.AluOpType.mult, mybir.AluOpType.add,
            )

            H = OUT_CHUNK // 2
            ob = cpool.tile([BATCH, OUT_CHUNK], mybir.dt.uint8)
            nc.vector.tensor_tensor(
                ob[:, :H], psum[:, :H], r2[:, :H], mybir.AluOpType.is_gt
            )
            nc.sync.dma_start(out=out_d[:, :H], in_=ob[:, :H])
            nc.vector.tensor_tensor(
                ob[:, H:], psum[:, H:], r2[:, H:], mybir.AluOpType.is_gt
            )
            nc.scalar.dma_start(out=out_d[:, H:], in_=ob[:, H:])

    nc.compile()
    return nc


def _install_ntff_hook_shim():
    """Provide antenv.axon_hooks (absent in this image) so trace=True works.

    Replicates trn_agent_boot's ctypes hook against libaxon_pjrt.so.
    """
    import sys

    if "antenv.axon_hooks" in sys.modules:
        return
    import contextlib
    import ctypes
    import types

    so_path = "/opt/axon/libaxon_pjrt.so"
    hook = None
    if os.path.exists(so_path):
        lib = ctypes.CDLL(so_path)
        if hasattr(lib, "axon_start_nrt_profile"):
            lib.axon_start_nrt_profile.argtypes = [
                ctypes.POINTER(ctypes.c_int64), ctypes.c_size_t,
            ]
            lib.axon_start_nrt_profile.restype = ctypes.c_int64
            lib.axon_stop_nrt_profile.argtypes = [ctypes.c_char_p]
            lib.axon_stop_nrt_profile.restype = ctypes.c_int64

            @contextlib.contextmanager
            def _hook(output_dir, device_ids):
                import jax
                jax.devices()
                if device_ids:
                    ids = (ctypes.c_int64 * len(device_ids))(*device_ids)
                    rc = lib.axon_start_nrt_profile(ids, len(device_ids))
                else:
                    rc = lib.axon_start_nrt_profile(None, 0)
                if rc != 0:
                    raise RuntimeError(f"axon_start_nrt_profile rc={rc}")
                try:
                    yield
                finally:
                    n = lib.axon_stop_nrt_profile(str(output_dir).encode())
                    print(f"ntff profile: {n} file(s) -> {output_dir}", file=sys.stderr)

            hook = _hook

    mod = types.ModuleType("antenv.axon_hooks")
    mod.get_axon_ntff_profile_hook = lambda: hook
    mod.set_axon_ntff_profile_hook = lambda h: None
    sys.modules["antenv.axon_hooks"] = mod


def kernel(x, masks, thresholds):
    global _nc, last_results
    from concourse.bass_utils import run_bass_kernel_spmd

    if bool(int(os.environ.get("KERNEL_TRACE", "0"))):
        _install_ntff_hook_shim()

    if _nc is None:
        _nc = _build()

    fp8 = ml_dtypes.float8_e4m3fn
    x_u8 = np.ascontiguousarray(np.asarray(x), dtype=np.uint8)
    m_u8 = np.asarray(masks)
    if m_u8.dtype != np.uint8:
        m_u8 = m_u8.astype(np.uint8)
    thr = np.asarray(thresholds, dtype=np.int32)

    # weights: (2x-1) encoded as fp8 bytes 0x01 (+2^-9) / 0x81 (-2^-9),
    # laid out as [partition, k-tile, batch]
    sign = np.where(x_u8.T != 0, np.uint8(0x01), np.uint8(0x81))  # [DIM_IN, B]
    xt = np.ascontiguousarray(
        sign.reshape(K_TILES, 128, BATCH).transpose(1, 0, 2)
    ).reshape(128, K_TILES * BATCH)

    # pre-tile all mask slices to partition-major in one pass:
    # [DIM_IN, DIM_OUT] -> per core [128, K_TILES * OUT_CHUNK] where
    # row p holds masks[k*128 + p, o0 + c] at free offset k*512 + c
    m_t = np.ascontiguousarray(
        m_u8.reshape(K_TILES, 128, N_CORES, OUT_CHUNK).transpose(2, 1, 0, 3)
    )  # [core, 128, K_TILES, OUT_CHUNK]

    in_maps = []
    for c in range(N_CORES):
        sl = slice(c * OUT_CHUNK, (c + 1) * OUT_CHUNK)
        mk = np.hstack([xt, m_t[c].reshape(128, K_TILES * OUT_CHUNK)])
        in_maps.append({
            "mk": mk.view(fp8),
            "tb": np.ascontiguousarray(
                np.broadcast_to(thr[sl][None, :], (BATCH, OUT_CHUNK))
            ),
        })

    last_results = run_bass_kernel_spmd(
        _nc, in_maps, core_ids=list(range(N_CORES)),
        trace=bool(int(os.environ.get("KERNEL_TRACE", "0"))),
    )
    out = np.concatenate([r["out"] for r in last_results.results], axis=1)
    return out.astype(np.bool_)


# revision 29
# speedup vs baseline: 1.0595x; 1.0595x over previous
"""Trainium2 Bass kernel for nn_Block_11020886082299.

Computes, for x: bool[B, DIM_IN], masks: bool[DIM_IN, DIM_OUT],
thresholds: int32[DIM_OUT]:

    sums[b, o] = sum_i XNOR(x[b, i], masks[i, o])
    out[b, o]  = sums[b, o] > thresholds[o]

Math used on device (all exact in fp32):

    sums = DIM_IN - sx[b] - sm[o] + 2 * (x @ m)      (sx/sm = row/col sums)

Encode the x-side weights as (2x-1) in {-1, +1} and stream the raw mask
bytes (0x00 / 0x01) directly as fp8e4m3 (0x01 == 2^-9 denormal, handled
exactly by the PE):

    psum[b, o] = sum_i (2x-1)*U * m*U = U^2 * (2*mm - sm),   U = 2^-9

so with r2[b, o] = U^2 * (t[o] - DIM_IN + sx[b]):

    out = psum > r2   <=>  2*mm - sm > t - DIM_IN + sx  <=>  sums > t

No elementwise conversion of the 16 MB mask tensor ever happens: the DMA
moves raw bytes, the PE consumes them as fp8.

Sharding: tensor-parallel over DIM_OUT across 8 cores (512 columns each);
x is replicated. This minimizes total HBM traffic (each core reads only
its 2 MB slice of masks).
"""

import os

import numpy as np
import ml_dtypes

BATCH = 64
DIM_IN = 4096
DIM_OUT = 4096
N_CORES = 8
OUT_CHUNK = DIM_OUT // N_CORES  # 512
K_TILES = DIM_IN // 128  # 32
CHUNK = 8  # k-tiles per mask DMA (8 * 64KB = 512KB per transfer)
N_CHUNKS = K_TILES // CHUNK
U2 = 2.0 ** -18  # (2^-9)^2 — scale of all PSUM values

_nc = None
last_results = None


def _build():
    import concourse.mybir as mybir
    from concourse import bacc
    from concourse.tile import TileContext

    FP8 = mybir.dt.float8e4
    F32 = mybir.dt.float32
    nc = bacc.Bacc(None, target_bir_lowering=False, debug=False)

    # Combined weights + masks tensor, partition-major so every DMA
    # descriptor is a contiguous multi-KB run per partition:
    #   bytes [0, 2048):       xt[p, k*64+b] = (2x-1)*U   (x side, host-tiled)
    #   bytes [2048, 18432):   mk[p, k*512+c] = raw mask bytes as fp8
    XT_W = K_TILES * BATCH  # 2048
    mk_d = nc.dram_tensor(
        "mk", [128, XT_W + K_TILES * OUT_CHUNK], FP8, kind="ExternalInput"
    )
    # thresholds chunk, broadcast to BATCH rows on host
    tb_d = nc.dram_tensor("tb", [BATCH, OUT_CHUNK], mybir.dt.int32, kind="ExternalInput")
    out_d = nc.dram_tensor("out", [BATCH, OUT_CHUNK], mybir.dt.uint8, kind="ExternalOutput")

    N_WARM = 5  # matmuls to lift the PE HAM clock gate before real data lands

    with TileContext(nc) as tc:
        with (
            tc.tile_pool(name="const", bufs=1) as cpool,
            tc.tile_pool(name="mkp", bufs=1) as mpool,
            tc.tile_pool(name="ps", bufs=1, space="PSUM") as pspool,
        ):
            # ---- warmup memsets first on gpsimd (they gate the PE start).
            # HAM watches MAC activity, so warmup operands must be nonzero:
            # +1 on the top half of K, -1 on the bottom half, so every
            # accumulation cancels exactly to 0.0 in fp32.
            warm = cpool.tile([128, OUT_CHUNK + BATCH], FP8)
            nc.gpsimd.memset(warm[:64, :OUT_CHUNK], 1.0)
            nc.gpsimd.memset(warm[64:, :OUT_CHUNK], -1.0)
            nc.gpsimd.memset(warm[:, OUT_CHUNK:], 2.0 ** -9)
            # ---- stream [xt | masks] in ramped chunks over the two HWDGE
            # rings (the gpsimd SWDGE ring lags ~2.5us, so it only gets the
            # final chunk).
            K_LO = [0, 2, 5, 9, 14, 20, 26, 32]
            RINGS = ["sync", "scalar", "sync", "scalar", "sync", "scalar", "gpsimd"]
            NCH = len(K_LO) - 1
            bounds = [0] + [XT_W + K_LO[i + 1] * OUT_CHUNK for i in range(NCH)]
            mts = []
            for c in range(NCH):
                mt = mpool.tile(
                    [128, bounds[c + 1] - bounds[c]], FP8, tag=f"mk{c}"
                )
                eng = getattr(nc, RINGS[c])
                eng.dma_start(out=mt[:, :], in_=mk_d[:, bounds[c]:bounds[c + 1]])
                mts.append(mt)
            xt_sb = mts[0]  # xt lives in chunk 0, bytes [0, XT_W)

            ones_f = cpool.tile([128, 1], F32)
            nc.gpsimd.memset(ones_f[:, :], 1.0)
            tb_b = cpool.tile([BATCH, OUT_CHUNK], mybir.dt.int32)
            nc.gpsimd.dma_start(out=tb_b[:, :], in_=tb_d[:, :])

            def rhs_for(k):
                c = next(i for i in range(NCH) if K_LO[i] <= k < K_LO[i + 1])
                off = (XT_W if c == 0 else 0) + (k - K_LO[c]) * OUT_CHUNK
                return mts[c][:, off:off + OUT_CHUNK]

            # ---- sx via on-chip data only: xsum[p, b] = sum_k xt[p, k*64+b]
            # (DVE strided reduce over xt, which is already in SBUF), then a
            # single fp32 ones-matmul reduces over partitions:
            #   psx[b] = sum_p xsum[p, b] = U * (2*sx[b] - DIM_IN)
            xsum = cpool.tile([128, BATCH], F32)
            xt3 = xt_sb[:, :XT_W].rearrange("p (k b) -> p b k", b=BATCH)
            nc.vector.tensor_reduce(
                xsum[:, :], xt3, axis=mybir.AxisListType.X, op=mybir.AluOpType.add
            )
            psx = pspool.tile([BATCH, 1], F32, tag="psx")

            psum = pspool.tile([BATCH, OUT_CHUNK], F32)
            for w in range(N_WARM):
                nc.tensor.matmul(
                    psum[:, :], warm[:, OUT_CHUNK:], warm[:, :OUT_CHUNK],
                    start=(w == 0), stop=False, skip_group_check=True,
                )
            for k in range(K_TILES):
                nc.tensor.matmul(
                    psum[:, :],
                    xt_sb[:, k * BATCH:(k + 1) * BATCH],
                    rhs_for(k),
                    start=False,
                    stop=(k == K_TILES - 1),
                )
                if k == 12:
                    nc.tensor.matmul(
                        psx[:, :], xsum[:, :], ones_f[:, :], start=True, stop=True
                    )

            # sxb = U^2*(sx - DIM_IN);  r2 = U^2*t + sxb — ready mid-stream
            sxb = cpool.tile([BATCH, 1], F32)
            nc.vector.tensor_scalar(
                sxb[:, :], psx[:, :], 2.0 ** -10, -float(DIM_IN) / 2.0 * U2,
                mybir.AluOpType.mult, mybir.AluOpType.add,
            )
            r2 = cpool.tile([BATCH, OUT_CHUNK], F32)
            nc.vector.tensor_scalar(
                r2[:, :], tb_b[:, :], U2, sxb[:, 0:1],
                mybir.AluOpType.mult, mybir.AluOpType.add,
            )

            ob = cpool.tile([BATCH, OUT_CHUNK], mybir.dt.uint8)
            nc.vector.tensor_tensor(ob[:, :], psum[:, :], r2[:, :], mybir.AluOpType.is_gt)
            nc.sync.dma_start(out=out_d[:32, :], in_=ob[:32, :])
            nc.scalar.dma_start(out=out_d[32:, :], in_=ob[32:, :])

    nc.compile()
    return nc


def _install_ntff_hook_shim():
    """Provide antenv.axon_hooks (absent in this image) so trace=True works.

    Replicates trn_agent_boot's ctypes hook against libaxon_pjrt.so.
    """
    import sys

    if "antenv.axon_hooks" in sys.modules:
        return
    import contextlib
    import ctypes
    import types

    so_path = "/opt/axon/libaxon_pjrt.so"
    hook = None
    if os.path.exists(so_path):
        lib = ctypes.CDLL(so_path)
        if hasattr(lib, "axon_start_nrt_profile"):
            lib.axon_start_nrt_profile.argtypes = [
                ctypes.POINTER(ctypes.c_int64), ctypes.c_size_t,
            ]
            lib.axon_start_nrt_profile.restype = ctypes.c_int64
            lib.axon_stop_nrt_profile.argtypes = [ctypes.c_char_p]
            lib.axon_stop_nrt_profile.restype = ctypes.c_int64

            @contextlib.contextmanager
            def _hook(output_dir, device_ids):
                import jax
                jax.devices()
                if device_ids:
                    ids = (ctypes.c_int64 * len(device_ids))(*device_ids)
                    rc = lib.axon_start_nrt_profile(ids, len(device_ids))
                else:
                    rc = lib.axon_start_nrt_profile(None, 0)
                if rc != 0:
                    raise RuntimeError(f"axon_start_nrt_profile rc={rc}")
                try:
                    yield
                finally:
                    n = lib.axon_stop_nrt_profile(str(output_dir).encode())
                    print(f"ntff profile: {n} file(s) -> {output_dir}", file=sys.stderr)

            hook = _hook

    mod = types.ModuleType("antenv.axon_hooks")
    mod.get_axon_ntff_profile_hook = lambda: hook
    mod.set_axon_ntff_profile_hook = lambda h: None
    sys.modules["antenv.axon_hooks"] = mod


def kernel(x, masks, thresholds):
    global _nc, last_results
    from concourse.bass_utils import run_bass_kernel_spmd

    if bool(int(os.environ.get("KERNEL_TRACE", "0"))):
        _install_ntff_hook_shim()

    if _nc is None:
        _nc = _build()

    fp8 = ml_dtypes.float8_e4m3fn
    x_u8 = np.ascontiguousarray(np.asarray(x), dtype=np.uint8)
    m_u8 = np.asarray(masks)
    if m_u8.dtype != np.uint8:
        m_u8 = m_u8.astype(np.uint8)
    thr = np.asarray(thresholds, dtype=np.int32)

    # weights: (2x-1) encoded as fp8 bytes 0x01 (+2^-9) / 0x81 (-2^-9),
    # laid out as [partition, k-tile, batch]
    sign = np.where(x_u8.T != 0, np.uint8(0x01), np.uint8(0x81))  # [DIM_IN, B]
    xt = np.ascontiguousarray(
        sign.reshape(K_TILES, 128, BATCH).transpose(1, 0, 2)
    ).reshape(128, K_TILES * BATCH)

    # pre-tile all mask slices to partition-major in one pass:
    # [DIM_IN, DIM_OUT] -> per core [128, K_TILES * OUT_CHUNK] where
    # row p holds masks[k*128 + p, o0 + c] at free offset k*512 + c
    m_t = np.ascontiguousarray(
        m_u8.reshape(K_TILES, 128, N_CORES, OUT_CHUNK).transpose(2, 1, 0, 3)
    )  # [core, 128, K_TILES, OUT_CHUNK]

    in_maps = []
    for c in range(N_CORES):
        sl = slice(c * OUT_CHUNK, (c + 1) * OUT_CHUNK)
        mk = np.hstack([xt, m_t[c].reshape(128, K_TILES * OUT_CHUNK)])
        in_maps.append({
            "mk": mk.view(fp8),
            "tb": np.ascontiguousarray(
                np.broadcast_to(thr[sl][None, :], (BATCH, OUT_CHUNK))
            ),
        })

    last_results = run_bass_kernel_spmd(
        _nc, in_maps, core_ids=list(range(N_CORES)),
        trace=bool(int(os.environ.get("KERNEL_TRACE", "0"))),
    )
    out = np.concatenate([r["out"] for r in last_results.results], axis=1)
    return out.astype(np.bool_)
